# revision 10
# baseline (speedup 1.0000x reference)
"""MoE soft-routing MLP kernel for 8 Trainium2 NeuronCores.

Reference computation (per layer l, weights a_l: [E, out, in], bias b_l: [E, out]):
    y_e = H @ a_e^T + b_e          # per-expert GEMM      [B, out]
    H'  = sum_e wb[e, :, None] * y_e                      [B, out]
    H'  = elu(H') for layers 0, 1

Distribution: data-parallel over batch B=4096 across 8 cores (B_loc=512).
Expert weights are replicated to every core; x and weight_blend are sharded
along batch.

Per-core algorithm (all activations kept TRANSPOSED on chip: [feature, batch]):
    out[o, b] = sum_e sum_i aT_e[i, o] * (wb[e, b] * Ht[i, b])  + sum_e b_e[o] wb[e, b]
  - the bias term is a K=8 matmul (lhsT = beta [E, o-chunk], rhs = wb [E, b])
    that seeds each PSUM bank (start=True),
  - each expert's contribution accumulates into the same PSUM bank:
    lhsT = aT_e[i-tile, o-chunk] (128x128 stationary),
    rhs  = Zt_e[i-tile] = Ht[i-tile] * bcast(wb[e, :]) (128x512 moving),
  - blend weights arrive pre-broadcast from the host ([E, 128, B_LOC]).
  - ELU+1 is evicted as relu(x) + min(exp(x), 1)  (= elu(x) + 1; valid since
    the preactivations here are far below exp-overflow), and the -1 folds
    into the next layer's blend: zt = (h1 - 1) * wbb_e, one DVE op.

Matmuls run in float32r (TF32-like, 1 cycle/row at N=512 vs 4 for fp32;
measured rel-err ~1.6e-4 per K=128 matmul). Everything else is fp32.

The output of the final layer is DMA'd out still transposed ([512, 512] per
core) and un-transposed on the host.
"""

import os
import sys

if "/opt/trn_rl_repo" not in sys.path:
    sys.path.insert(0, "/opt/trn_rl_repo")

import numpy as np

import concourse.bass as bass  # noqa: F401  (bass must import before mybir use)
import concourse.mybir as mybir
import concourse.tile as tile
from concourse import bacc
from concourse.bass_utils import run_bass_kernel_spmd

F32 = mybir.dt.float32
F32R = mybir.dt.float32r
AF = mybir.ActivationFunctionType
ALU = mybir.AluOpType

B, E = 4096, 8
DIMS = [512, 1024, 1024, 512]
N_CORES = 8
B_LOC = B // N_CORES  # 512; also the matmul moving free-dim (max for 4-byte)
P = 128

# (in, out, apply_elu) per layer
LAYERS = [
    (DIMS[0], DIMS[1], True),
    (DIMS[1], DIMS[2], True),
    (DIMS[2], DIMS[3], False),
]

LAST_RESULTS = None  # BassKernelResults of the most recent run (for test.py)
_NC_CACHE = None


def _build():
    nc = bacc.Bacc(None, target_bir_lowering=False, debug=False)

    xt = nc.dram_tensor("xt", [DIMS[0], B_LOC], F32, kind="ExternalInput")
    wb = nc.dram_tensor("wb", [E, B_LOC], F32R, kind="ExternalInput")
    wbbd = nc.dram_tensor("wbb", [E, P, B_LOC], F32, kind="ExternalInput")
    ats = [
        nc.dram_tensor(f"a{l}t", [E, din, dout], F32R, kind="ExternalInput")
        for l, (din, dout, _) in enumerate(LAYERS)
    ]
    betas = [
        nc.dram_tensor(f"b{l}", [E, dout], F32R, kind="ExternalInput")
        for l, (_, dout, _) in enumerate(LAYERS)
    ]
    outt = nc.dram_tensor("outt", [DIMS[3], B_LOC], F32, kind="ExternalOutput")

    with tile.TileContext(nc) as tc:
        with (
            tc.tile_pool(name="htp", bufs=12) as htp,
            tc.tile_pool(name="ztp", bufs=8) as ztp,
            tc.tile_pool(name="wp", bufs=8) as wp,
            tc.tile_pool(name="wbbp", bufs=8) as wbbp,
            tc.tile_pool(name="consts", bufs=1) as consts,
            tc.tile_pool(name="betap", bufs=2) as betap,
            tc.tile_pool(name="tmp", bufs=3) as tmp,
            tc.tile_pool(name="psp", bufs=8, space="PSUM") as psp,
        ):
            # --- startup: x^T tiles, blend weights (pre-broadcast on host) ---
            ht = []
            for j in range(DIMS[0] // P):
                t = htp.tile([P, B_LOC], F32, tag="ht")
                nc.sync.dma_start(out=t, in_=xt[j * P : (j + 1) * P, :])
                ht.append(t)
            # wb as [E, B_LOC] tile: rhs of the bias matmuls
            wb_all = consts.tile([E, B_LOC], F32R, tag="wb_all")
            nc.sync.dma_start(out=wb_all, in_=wb[:, :])
            wbb = []
            for e in range(E):
                t = wbbp.tile([P, B_LOC], F32, tag="wbb")
                nc.sync.dma_start(out=t, in_=wbbd[e])
                wbb.append(t)

            # --- layers ---
            for l, (din, dout, use_act) in enumerate(LAYERS):
                ni, no = din // P, dout // P
                beta_sb = betap.tile([E, dout], F32R, tag="beta")
                nc.sync.dma_start(out=beta_sb, in_=betas[l][:, :])

                # seed each PSUM bank with the blended bias (K=8 matmul)
                psums = []
                for c in range(no):
                    pt = psp.tile([P, B_LOC], F32, tag="ps")
                    nc.tensor.matmul(
                        pt,
                        beta_sb[:, c * P : (c + 1) * P],
                        wb_all,
                        start=True,
                        stop=False,
                    )
                    psums.append(pt)

                # accumulate all experts
                for e in range(E):
                    for j in range(ni):
                        zt = ztp.tile([P, B_LOC], F32R, tag="zt")
                        if l == 0:
                            nc.vector.tensor_mul(zt, ht[j], wbb[e])
                        else:
                            # ht holds elu(x)+1; fold the -1 into the blend
                            nc.vector.scalar_tensor_tensor(
                                zt, ht[j], -1.0, wbb[e], ALU.add, ALU.mult
                            )
                        at_sb = wp.tile([P, dout], F32R, tag="w")
                        nc.sync.dma_start(
                            out=at_sb, in_=ats[l][e, j * P : (j + 1) * P, :]
                        )
                        last = e == E - 1 and j == ni - 1
                        for c in range(no):
                            nc.tensor.matmul(
                                psums[c],
                                at_sb[:, c * P : (c + 1) * P],
                                zt,
                                start=False,
                                stop=last,
                            )

                # evict: elu(x)+1 for layers 0/1, direct DMA out for layer 2
                if use_act:
                    new_ht = []
                    for c in range(no):
                        r = tmp.tile([P, B_LOC], F32, tag="relu")
                        x = tmp.tile([P, B_LOC], F32, tag="expz")
                        h = htp.tile([P, B_LOC], F32, tag="ht")
                        nc.scalar.activation(r, psums[c], AF.Relu)
                        nc.scalar.activation(x, psums[c], AF.Exp)
                        # h = min(x, 1) + r  ( = elu + 1 )
                        nc.vector.scalar_tensor_tensor(h, x, 1.0, r, ALU.min, ALU.add)
                        new_ht.append(h)
                    ht = new_ht
                else:
                    for c in range(no):
                        o = tmp.tile([P, B_LOC], F32, tag="out")
                        nc.scalar.activation(o, psums[c], AF.Copy)
                        nc.sync.dma_start(out=outt[c * P : (c + 1) * P, :], in_=o)

    nc.compile()
    return nc


def kernel(x, weight_blend, a0, b0, a1, b1, a2, b2):
    global LAST_RESULTS, _NC_CACHE
    x = np.ascontiguousarray(np.asarray(x, dtype=np.float32))
    weight_blend = np.ascontiguousarray(np.asarray(weight_blend, dtype=np.float32))
    aT = [
        np.ascontiguousarray(np.asarray(a, dtype=np.float32).transpose(0, 2, 1))
        for a in (a0, a1, a2)
    ]
    bs = [np.ascontiguousarray(np.asarray(b, dtype=np.float32)) for b in (b0, b1, b2)]

    if _NC_CACHE is None:
        _NC_CACHE = _build()
    nc = _NC_CACHE

    in_maps = []
    for c in range(N_CORES):
        sl = slice(c * B_LOC, (c + 1) * B_LOC)
        wb_c = np.ascontiguousarray(weight_blend[:, sl])
        in_maps.append(
            {
                "xt": np.ascontiguousarray(x[sl].T),
                "wb": wb_c,
                "wbb": np.ascontiguousarray(
                    np.broadcast_to(wb_c[:, None, :], (E, P, B_LOC))
                ),
                "a0t": aT[0],
                "a1t": aT[1],
                "a2t": aT[2],
                "b0": bs[0],
                "b1": bs[1],
                "b2": bs[2],
            }
        )

    trace = os.environ.get("BASS_KERNEL_TRACE") == "1"
    res = run_bass_kernel_spmd(
        nc, in_maps, core_ids=list(range(N_CORES)), trace=trace
    )
    LAST_RESULTS = res
    return np.concatenate(
        [np.asarray(r["outt"]).T for r in res.results], axis=0
    ).astype(np.float32)
